# revision 35
# baseline (speedup 1.0000x reference)
"""Multi-head attention (B=2, T=2048, D=1024, H=16) on 8 NeuronCores.

Sharding: core c handles batch b=c//4 and head-group g=c%4 (4 heads = 256
of the 1024 e-dims). QKV weights are column-sharded, w_o row-sharded.
Each core returns a [T, D] partial of the output projection; the host sums
the 4 partials per batch and folds in b_v @ w_o^T + b_o.

Device algorithm (per core), all matmuls bf16 (full PE rate):
  Phase 1A: K-em0 (full T) and Q-em0 (t-block 0) projections.
  Then 8 passes, one per (head-pair hp, t-block tb), hp-major. Per pass,
  a 16-iteration s-tile loop: scores^T psum [s, 2-head, t] (2 matmuls,
  K'=64), one Exp activation -> pT bf16. P@V uses lhsT = [V_head | ones]
  (M=65): psum row 64 accumulates the softmax denominator for free
  (matmul cost is N cycles regardless of M). pv emission lags 9 slots so
  the previous pass's denominator reciprocal -> DRAM bounce -> partition
  broadcast -> normalize-mul completes before the pv banks are reused
  (pv is single-buffered). Remaining projections (V during pass 0,
  K-em1/Q chunks during passes 1-4) and the per-head output-projection
  units (4 K'=64 matmuls each, dripped after both head-pairs of a
  t-block are normalized) fill PE slack in the ACT-paced s-loops.

PSUM budget (8 banks): scores 2x2 (double-buffered head-pair tiles) +
pv 2 (single-buffered [65, 2, TB]) + 2 rotating (projection chunks /
V chunks / y units).
"""

import sys
from collections import deque
from contextlib import ExitStack

import numpy as np

try:
    import concourse.bass as bass
except ImportError:  # pragma: no cover
    sys.path.insert(0, "/opt/trn_rl_repo")
    import concourse.bass as bass

import concourse.tile as tile
from concourse import mybir
from concourse.bass_utils import run_bass_kernel_spmd

F32 = mybir.dt.float32
BF16 = mybir.dt.bfloat16

D = 1024
H = 16
DK = 64
E = 256  # per-core out-dim of the head group (4 heads x 64)
P = 128
N_CORES = 8
VW = 66  # V row stride per head: 64 data + 1 ones + 1 pad


def _split_multi_waits(nc):
    """This container's walrus encodes at most ONE sync-wait per instruction
    ("Too many sync wait commands" in codegen otherwise). Tile attaches
    multi-sem waits to instructions; hoist all but the last wait onto
    standalone single-wait EventSemaphore instructions inserted just before,
    on the same engine — semantically identical (engine stalls in order)."""
    n = 0
    for fn in nc.m.functions:
        for bb in fn.blocks:
            il = bb.instructions
            i = 0
            while i < len(il):
                ins = il[i]
                si = ins.sync_info
                if si is not None and si.on_wait and len(si.on_wait) > 1:
                    waits = list(si.on_wait)
                    for k, w in enumerate(waits[:-1]):
                        ev = mybir.InstEventSemaphore(
                            name=f"{ins.name}_w{k}", ins=[], outs=[],
                            sync_info=mybir.SyncInfo(on_wait=[w], on_update=[]),
                        )
                        ev.engine = ins.engine
                        nc.register_instruction(ev)
                        il.insert(i, ev)
                        i += 1
                        n += 1
                    si.on_wait = waits[-1:]
                i += 1
    return n


def build_nc(T=2048, TB=512):
    NT = T // P       # s-tiles (16)
    NTB = T // TB     # t-blocks (4)
    NPASS = 2 * NTB   # (head-pair, t-block) passes, hp-major
    LAG = 7           # pv emission lag in s-slots (> mul slot 6)

    nc = bass.Bass()

    xT_d = nc.dram_tensor("xT", [D, T], BF16, kind="ExternalInput")
    wqT_d = nc.dram_tensor("wqT", [D, E], BF16, kind="ExternalInput")
    wkT_d = nc.dram_tensor("wkT", [D, E], BF16, kind="ExternalInput")
    wvT_d = nc.dram_tensor("wvT", [D, E], BF16, kind="ExternalInput")
    wo_d = nc.dram_tensor("wo_sh", [E, D], BF16, kind="ExternalInput")
    id_d = nc.dram_tensor("ident", [DK, DK], BF16, kind="ExternalInput")
    bq_d = nc.dram_tensor("bq2", [P, 2], F32, kind="ExternalInput")
    bk_d = nc.dram_tensor("bk2", [P, 2], F32, kind="ExternalInput")
    y_d = nc.dram_tensor("y", [T, D], F32, kind="ExternalOutput")
    den_dram = nc.dram_tensor("den_scratch", [NPASS, 2, TB], F32)

    with tile.TileContext(nc) as tc:
        with (
            tc.tile_pool(name="sb", bufs=1) as sb,
            tc.tile_pool(name="ps", bufs=1, space="PSUM") as ps,
        ):
            # ---- persistent SBUF tiles (unique tags => unique slots) ----
            QT = sb.tile([P, 2, T], BF16, tag="QT", name="QT")
            KT = sb.tile([P, 2, T], BF16, tag="KT", name="KT")
            V = sb.tile([P, NT, 4, VW], BF16, tag="V", name="V")
            # outT packs a head-pair per em: even head at partitions 0-63,
            # odd head at 64-127, so the output projection contracts K'=128
            outT = sb.tile([P, 2, T], BF16, tag="outT", name="outT")
            wo_sb = sb.tile([P, 2, D], BF16, tag="wo", name="wo_sb")
            id_sb = sb.tile([DK, DK], BF16, tag="ident", name="id_sb")
            xT_sb = sb.tile([P, 8, T], BF16, tag="xT", name="xT_sb")
            wq_sb = sb.tile([P, 8, E], BF16, tag="wq", name="wq_sb")
            wk_sb = sb.tile([P, 8, E], BF16, tag="wk", name="wk_sb")
            wv_sb = sb.tile([P, 8, E], BF16, tag="wv", name="wv_sb")
            bq_sb = sb.tile([P, 2], F32, tag="bq", name="bq_sb")
            bk_sb = sb.tile([P, 2], F32, tag="bk", name="bk_sb")

            # ---- input DMAs, ordered for the phase-1A critical path: the
            # em0 halves of the K/Q weights land first so the first scores
            # column can start ~4us in ----
            nc.sync.dma_start(out=wk_sb[:, :, 0:P],
                              in_=wkT_d[:, 0:P].rearrange("(dt p) e -> p dt e", p=P))
            for dt in range(8):
                nc.sync.dma_start(out=xT_sb[:, dt, 0:512],
                                  in_=xT_d[dt * P:(dt + 1) * P, 0:512])
            nc.sync.dma_start(out=wq_sb[:, :, 0:P],
                              in_=wqT_d[:, 0:P].rearrange("(dt p) e -> p dt e", p=P))
            nc.sync.dma_start(out=wv_sb, in_=wvT_d[:].rearrange("(dt p) e -> p dt e", p=P))
            nc.sync.dma_start(out=bk_sb, in_=bk_d[:])
            nc.sync.dma_start(out=bq_sb, in_=bq_d[:])
            nc.sync.dma_start(out=wk_sb[:, :, P:E],
                              in_=wkT_d[:, P:E].rearrange("(dt p) e -> p dt e", p=P))
            nc.sync.dma_start(out=wq_sb[:, :, P:E],
                              in_=wqT_d[:, P:E].rearrange("(dt p) e -> p dt e", p=P))
            for c in range(1, T // 512):
                for dt in range(8):
                    nc.sync.dma_start(
                        out=xT_sb[:, dt, c * 512:(c + 1) * 512],
                        in_=xT_d[dt * P:(dt + 1) * P, c * 512:(c + 1) * 512],
                    )
            nc.sync.dma_start(out=id_sb, in_=id_d[:])
            nc.sync.dma_start(out=wo_sb, in_=wo_d[:].rearrange("(m p) f -> p m f", p=P))

            # ones columns of V (denominator rows of the pv matmuls)
            nc.vector.memset(V[:, :, :, 64:65], 1.0)

            # ---- drip-work helpers (each item is an atomic closure) ----
            def proj_half(w_sb, em, c, dst, b_sb, state, second):
                # half of a K/Q projection chunk: 4 matmuls (+ bias drain)
                def f():
                    if not second:
                        state["ps"] = ps.tile([P, 512], F32, tag="y", bufs=2,
                                              name="proj_ps")
                    for dt in (range(4, 8) if second else range(4)):
                        nc.tensor.matmul(
                            state["ps"],
                            lhsT=w_sb[:, dt, em * P:(em + 1) * P],
                            rhs=xT_sb[:, dt, c * 512:(c + 1) * 512],
                            start=(dt == 0),
                            stop=(dt == 7),
                            skip_group_check=True,
                        )
                    if second:
                        nc.vector.tensor_scalar_add(
                            out=dst[:, em, c * 512:(c + 1) * 512],
                            in0=state["ps"],
                            scalar1=b_sb[:, em:em + 1],
                        )
                return f

            def proj_chunk(w_sb, em, c, dst, b_sb):
                st = {}
                return [proj_half(w_sb, em, c, dst, b_sb, st, False),
                        proj_half(w_sb, em, c, dst, b_sb, st, True)]

            def v_chunk(st):
                # V projection for one 128-wide s-tile (all 4 heads)
                vps = ps.tile([P, 512], F32, tag="y", bufs=2, name="vps")
                for dt in range(8):
                    nc.tensor.matmul(
                        vps[:, :E],
                        lhsT=xT_sb[:, dt, st * P:(st + 1) * P],
                        rhs=wv_sb[:, dt, :],
                        start=(dt == 0),
                        stop=(dt == 7),
                        skip_group_check=True,
                    )
                nc.vector.tensor_copy(out=V[:, st, :, 0:64], in_=vps[:, :E])

            yA_ring = {}

            def y_half(tt, fb, em):
                # Every y unit is em-split: the em0 half (heads 0,1) runs
                # right after that head-pair's normalize — one pass before
                # em1 — parking its partial in a bf16 ring. This turns the
                # bursty 8-unit-per-2-passes y supply into a smooth 8 halves
                # per pass, and leaves only the small em1 half of the last
                # t-block trailing the final softmax bounce.
                def f():
                    yp = ps.tile([P, 512], F32, tag="y", bufs=2, name="y_ps")
                    nc.tensor.matmul(
                        yp,
                        lhsT=outT[:, em, tt * P:(tt + 1) * P],
                        rhs=wo_sb[:, em, fb * 512:(fb + 1) * 512],
                        start=True,
                        stop=True,
                        skip_group_check=True,
                    )
                    if em == 0:
                        ya = sb.tile([P, 512], BF16, tag="yA", bufs=12, name="ya")
                        nc.vector.tensor_copy(out=ya, in_=yp)
                        yA_ring[(tt, fb)] = ya
                    else:
                        ysb = sb.tile([P, 512], F32, tag="ysb", bufs=2, name="ysb")
                        nc.vector.tensor_add(ysb, yp, yA_ring.pop((tt, fb)))
                        nc.sync.dma_start(
                            out=y_d[tt * P:(tt + 1) * P, fb * 512:(fb + 1) * 512],
                            in_=ysb,
                        )
                return f

            def pump(q, n):
                for _ in range(n):
                    if not q:
                        return
                    q.popleft()()

            # ---- phase 1A: first K-em0 and Q-em0 chunks only; all other
            # projection chunks stream into the early passes just ahead of
            # their first use (deadlines noted) ----
            for f in proj_chunk(wk_sb, 0, 0, KT, bk_sb):
                f()
            for f in proj_chunk(wq_sb, 0, 0, QT, bq_sb):
                f()

            projq = deque()
            pass_proj = {
                0: (proj_chunk(wk_sb, 0, 1, KT, bk_sb)      # sc(p0,st4)
                    + proj_chunk(wq_sb, 1, 0, QT, bq_sb)    # p1
                    + proj_chunk(wk_sb, 0, 2, KT, bk_sb)    # sc(p0,st8)
                    + proj_chunk(wk_sb, 1, 0, KT, bk_sb)    # sc(p1,st0)
                    + proj_chunk(wk_sb, 0, 3, KT, bk_sb)),  # sc(p0,st12)
                1: (proj_chunk(wk_sb, 1, 1, KT, bk_sb)      # sc(p1,st4) JIT
                    + proj_chunk(wk_sb, 1, 2, KT, bk_sb)    # sc(p1,st8)
                    + proj_chunk(wk_sb, 1, 3, KT, bk_sb)    # sc(p1,st12)
                    + proj_chunk(wq_sb, 0, 1, QT, bq_sb)),  # p2
                2: proj_chunk(wq_sb, 1, 1, QT, bq_sb),      # p3
                3: proj_chunk(wq_sb, 0, 2, QT, bq_sb),      # p4
                4: proj_chunk(wq_sb, 1, 2, QT, bq_sb),      # p5
                5: proj_chunk(wq_sb, 0, 3, QT, bq_sb),      # p6
                6: proj_chunk(wq_sb, 1, 3, QT, bq_sb),      # p7
            }

            yq = deque()

            def emit_pv(info, st):
                for j in range(2):
                    nc.tensor.matmul(
                        info["pv"][0:65, j, :],
                        lhsT=V[:, st, 2 * info["hp"] + j, 0:65],
                        rhs=info["pT"][:, st, j, :],
                        start=(st == 0),
                        stop=(st == NT - 1),
                        skip_group_check=True,
                    )

            def norm(info):
                # 1/denominator rows -> DRAM -> partition-broadcast tiles.
                rec = sb.tile([65, 2, TB], F32, tag="rec", bufs=2, name="rec")
                nc.vector.reciprocal(out=rec[64:65, :, :], in_=info["pv"][64:65, :, :])
                nc.sync.dma_start(out=den_dram[info["p"], :, :], in_=rec[64:65, :, :])
                rep = sb.tile([DK, 2, TB], F32, tag="rep", bufs=2, name="rep")
                for j in range(2):
                    nc.sync.dma_start(
                        out=rep[:, j, :],
                        in_=den_dram[info["p"], j:j + 1, :].to_broadcast([DK, TB]),
                    )
                info["rep"] = rep

            def mul(info):
                t0 = info["t0"]
                # even head: normalize psum straight into outT rows 0-63
                nc.vector.tensor_mul(
                    outT[0:DK, info["hp"], t0:t0 + TB],
                    info["pv"][0:DK, 0, :],
                    info["rep"][:, 0, :],
                )
                # odd head: normalize into SBUF, then PE identity-shift to
                # outT rows 64-127 (DVE lanes can't cross partitions)
                ou = sb.tile([DK, TB], BF16, tag="ou", bufs=2, name="ou")
                nc.vector.tensor_mul(ou, info["pv"][0:DK, 1, :], info["rep"][:, 1, :])
                sh = ps.tile([P, TB], F32, tag="y", bufs=2, name="sh_ps")
                nc.tensor.matmul(sh[DK:P, :], lhsT=id_sb, rhs=ou,
                                 start=True, stop=True, skip_group_check=True)
                nc.vector.tensor_copy(
                    out=outT[DK:P, info["hp"], t0:t0 + TB], in_=sh[DK:P, :])

            # ---- main passes ----
            # Emission order per slot: exp(st) first, then pv/spill/drip
            # (ready PE work), then sc(st+1) LAST — the next slot's scores
            # matmul is the only instruction that may briefly block on the
            # exp double-buffer WAR, and putting it after the drip keeps the
            # strict-FIFO PE queue from stalling on it while ready work sits
            # behind it.
            def emit_sc(info, st):
                scp = ps.tile([P, 2, TB], F32, tag="sc", bufs=2, name="sc_ps")
                for j in range(2):
                    nc.tensor.matmul(
                        scp[:, j, :],
                        lhsT=KT[DK * j:DK * (j + 1), info["hp"],
                                st * P:(st + 1) * P],
                        rhs=QT[DK * j:DK * (j + 1), info["hp"],
                               info["t0"]:info["t0"] + TB],
                        start=True,
                        stop=True,
                    )
                info["sc"][st] = scp

            def append_y(info):
                # called right after mul(info): that pass's heads are now
                # normalized in outT
                for tt in range(info["tb"] * (TB // P),
                                (info["tb"] + 1) * (TB // P)):
                    for fb in range(2):
                        yq.append(y_half(tt, fb, info["hp"]))

            prev = None
            for p in range(NPASS + 1):
                last = p == NPASS
                if not last:
                    tb, hp = p // 2, p % 2  # t-block-major: y work starts early
                    cur = {
                        "p": p, "hp": hp, "tb": tb, "t0": tb * TB,
                        "pT": sb.tile([P, NT, 2, TB], BF16, tag="pT", bufs=2,
                                      name="pT"),
                        "pv": ps.tile([65, 2, TB], F32, tag="pv", bufs=1,
                                      name="pv_ps"),
                        "pending": deque(range(NT - LAG, NT)),
                        "sc": {},
                    }
                    projq.extend(pass_proj.get(p, []))
                    emit_sc(cur, 0)

                for st in range(0 if last else NT):
                    nc.scalar.activation(
                        out=cur["pT"][:, st, :, :],
                        in_=cur["sc"].pop(st),
                        func=mybir.ActivationFunctionType.Exp,
                        scale=0.125,
                    )
                    if st + 1 < NT:
                        emit_sc(cur, st + 1)
                    if p == 0:
                        v_chunk(st)
                    if prev is not None:
                        if st <= 2:  # spill: previous pass's lagged pv tail
                            for _ in range(3):
                                if prev["pending"]:
                                    emit_pv(prev, prev["pending"].popleft())
                        elif st == 3:
                            norm(prev)
                        elif st == 6:
                            mul(prev)
                            append_y(prev)
                    if st >= LAG:
                        emit_pv(cur, st - LAG)
                    # drip pacing: passes 0-1 pump projections every slot
                    # (their queues carry K chunks with hard in-pass
                    # deadlines); later passes pump their single chunk at
                    # fixed slots. y halves fill the pv-free slots from the
                    # standing backlog, under-pumped on the final pass so a
                    # backlog rides into the tail's bounce window.
                    if p <= 1 or st in (5, 13):
                        pump(projq, 1)
                    if p >= 2 and (3 <= st <= 6 or
                                   (st % 2 == 1 and p != NPASS - 1)):
                        pump(yq, 1)

                if last:
                    # tail: finish pass 7's pv and get the denominator
                    # bounce into the DMA queue BEFORE the y-backlog's
                    # output DMAs, then burn the backlog during the bounce
                    while prev["pending"]:
                        emit_pv(prev, prev["pending"].popleft())
                    norm(prev)
                    pump(yq, len(yq))  # ready leftovers cover the bounce
                    mul(prev)
                    append_y(prev)
                    pump(projq, len(projq))
                    pump(yq, len(yq))
                else:
                    prev = cur
    _split_multi_waits(nc)
    return nc


def _shard_inputs(x, w_q, b_q, w_k, b_k, w_v, b_v, w_o, b_o):
    from ml_dtypes import bfloat16

    def bf(a):
        return np.ascontiguousarray(a).astype(bfloat16)

    in_maps = []
    for c in range(N_CORES):
        b, g = c // 4, c % 4
        sl = slice(g * E, (g + 1) * E)
        in_maps.append({
            "ident": np.eye(64, dtype=bfloat16),
            "xT": bf(x[b].T),
            "wqT": bf(w_q[sl, :].T),
            "wkT": bf(w_k[sl, :].T),
            "wvT": bf(w_v[sl, :].T),
            "wo_sh": bf(w_o[:, sl].T),
            "bq2": np.ascontiguousarray(b_q[sl].reshape(2, P).T, dtype=np.float32),
            "bk2": np.ascontiguousarray(b_k[sl].reshape(2, P).T, dtype=np.float32),
        })
    return in_maps


_NC_CACHE = {}


def kernel(x, w_q, b_q, w_k, b_k, w_v, b_v, w_o, b_o, _trace=False):
    x = np.asarray(x, dtype=np.float32)
    B, T, _ = x.shape
    args = [np.asarray(a, dtype=np.float32)
            for a in (w_q, b_q, w_k, b_k, w_v, b_v, w_o, b_o)]
    w_q, b_q, w_k, b_k, w_v, b_v, w_o, b_o = args

    if T not in _NC_CACHE:
        _NC_CACHE[T] = build_nc(T=T)
    nc = _NC_CACHE[T]
    in_maps = _shard_inputs(x, w_q, b_q, w_k, b_k, w_v, b_v, w_o, b_o)
    res = run_bass_kernel_spmd(nc, in_maps, list(range(N_CORES)), trace=_trace)

    y = np.zeros((B, T, D), dtype=np.float32)
    for c in range(N_CORES):
        y[c // 4] += res.results[c]["y"]
    fold = b_v @ w_o.T + b_o
    y += fold[None, None, :]
    if _trace:
        return y, res
    return y


# revision 44
# speedup vs baseline: 1.0450x; 1.0450x over previous
"""Multi-head attention (B=2, T=2048, D=1024, H=16) on 8 NeuronCores.

Sharding: core c handles batch b=c//4 and head-group g=c%4 (4 heads = 256
of the 1024 e-dims). QKV weights are column-sharded, w_o row-sharded.
Each core returns a [T, D] partial of the output projection; the host sums
the 4 partials per batch and folds in b_v @ w_o^T + b_o.

Device algorithm (per core), all matmuls bf16 (full PE rate):
  Phase 1A: K-em0 (full T) and Q-em0 (t-block 0) projections.
  Then 8 passes, one per (head-pair hp, t-block tb), hp-major. Per pass,
  a 16-iteration s-tile loop: scores^T psum [s, 2-head, t] (2 matmuls,
  K'=64), one Exp activation -> pT bf16. P@V uses lhsT = [V_head | ones]
  (M=65): psum row 64 accumulates the softmax denominator for free
  (matmul cost is N cycles regardless of M). pv emission lags 9 slots so
  the previous pass's denominator reciprocal -> DRAM bounce -> partition
  broadcast -> normalize-mul completes before the pv banks are reused
  (pv is single-buffered). Remaining projections (V during pass 0,
  K-em1/Q chunks during passes 1-4) and the per-head output-projection
  units (4 K'=64 matmuls each, dripped after both head-pairs of a
  t-block are normalized) fill PE slack in the ACT-paced s-loops.

PSUM budget (8 banks): scores 2x2 (double-buffered head-pair tiles) +
pv 2 (single-buffered [65, 2, TB]) + 2 rotating (projection chunks /
V chunks / y units).
"""

import sys
from collections import deque
from contextlib import ExitStack

import numpy as np

try:
    import concourse.bass as bass
except ImportError:  # pragma: no cover
    sys.path.insert(0, "/opt/trn_rl_repo")
    import concourse.bass as bass

import concourse.tile as tile
from concourse import mybir
from concourse.bass_utils import run_bass_kernel_spmd

F32 = mybir.dt.float32
F32R = mybir.dt.float32r
BF16 = mybir.dt.bfloat16

D = 1024
H = 16
DK = 64
E = 256  # per-core out-dim of the head group (4 heads x 64)
P = 128
N_CORES = 8
VW = 66  # V row stride per head: 64 data + 1 ones + 1 pad


def _split_multi_waits(nc):
    """This container's walrus encodes at most ONE sync-wait per instruction
    ("Too many sync wait commands" in codegen otherwise). Tile attaches
    multi-sem waits to instructions; hoist all but the last wait onto
    standalone single-wait EventSemaphore instructions inserted just before,
    on the same engine — semantically identical (engine stalls in order)."""
    n = 0
    for fn in nc.m.functions:
        for bb in fn.blocks:
            il = bb.instructions
            i = 0
            while i < len(il):
                ins = il[i]
                si = ins.sync_info
                if si is not None and si.on_wait and len(si.on_wait) > 1:
                    waits = list(si.on_wait)
                    for k, w in enumerate(waits[:-1]):
                        ev = mybir.InstEventSemaphore(
                            name=f"{ins.name}_w{k}", ins=[], outs=[],
                            sync_info=mybir.SyncInfo(on_wait=[w], on_update=[]),
                        )
                        ev.engine = ins.engine
                        nc.register_instruction(ev)
                        il.insert(i, ev)
                        i += 1
                        n += 1
                    si.on_wait = waits[-1:]
                i += 1
    return n


def build_nc(T=2048, TB=512):
    NT = T // P       # s-tiles (16)
    NTB = T // TB     # t-blocks (4)
    NPASS = 2 * NTB   # (head-pair, t-block) passes, hp-major
    LAG = 5           # pv emission lag in s-slots (> mul slot 4)

    nc = bass.Bass()

    xT_d = nc.dram_tensor("xT", [D, T], BF16, kind="ExternalInput")
    wqT_d = nc.dram_tensor("wqT", [D, E], BF16, kind="ExternalInput")
    wkT_d = nc.dram_tensor("wkT", [D, E], BF16, kind="ExternalInput")
    wvT_d = nc.dram_tensor("wvT", [D, E], BF16, kind="ExternalInput")
    wo_d = nc.dram_tensor("wo_sh", [E, D], BF16, kind="ExternalInput")
    id_d = nc.dram_tensor("ident", [DK, DK], BF16, kind="ExternalInput")
    bq_d = nc.dram_tensor("bq2", [P, 2], F32, kind="ExternalInput")
    bk_d = nc.dram_tensor("bk2", [P, 2], F32, kind="ExternalInput")
    y_d = nc.dram_tensor("y", [T, D], F32, kind="ExternalOutput")

    with tile.TileContext(nc) as tc:
        with (
            tc.tile_pool(name="sb", bufs=1) as sb,
            tc.tile_pool(name="ps", bufs=1, space="PSUM") as ps,
        ):
            # ---- persistent SBUF tiles (unique tags => unique slots) ----
            QT = sb.tile([P, 2, T], BF16, tag="QT", name="QT")
            KT = sb.tile([P, 2, T], BF16, tag="KT", name="KT")
            V = sb.tile([P, NT, 4, VW], BF16, tag="V", name="V")
            # outT packs a head-pair per em: even head at partitions 0-63,
            # odd head at 64-127, so the output projection contracts K'=128
            outT = sb.tile([P, 2, T], BF16, tag="outT", name="outT")
            wo_sb = sb.tile([P, 2, D], BF16, tag="wo", name="wo_sb")
            id_sb = sb.tile([DK, DK], BF16, tag="ident", name="id_sb")
            ones_bc = sb.tile([65, DK], F32R, tag="ones", name="ones_bc")
            xT_sb = sb.tile([P, 8, T], BF16, tag="xT", name="xT_sb")
            wq_sb = sb.tile([P, 8, E], BF16, tag="wq", name="wq_sb")
            wk_sb = sb.tile([P, 8, E], BF16, tag="wk", name="wk_sb")
            wv_sb = sb.tile([P, 8, E], BF16, tag="wv", name="wv_sb")
            bq_sb = sb.tile([P, 2], F32, tag="bq", name="bq_sb")
            bk_sb = sb.tile([P, 2], F32, tag="bk", name="bk_sb")

            # ---- input DMAs, ordered for the phase-1A critical path: the
            # em0 halves of the K/Q weights land first so the first scores
            # column can start ~4us in ----
            nc.sync.dma_start(out=wk_sb[:, :, 0:P],
                              in_=wkT_d[:, 0:P].rearrange("(dt p) e -> p dt e", p=P))
            for dt in range(8):
                nc.sync.dma_start(out=xT_sb[:, dt, 0:512],
                                  in_=xT_d[dt * P:(dt + 1) * P, 0:512])
            nc.sync.dma_start(out=wq_sb[:, :, 0:P],
                              in_=wqT_d[:, 0:P].rearrange("(dt p) e -> p dt e", p=P))
            nc.sync.dma_start(out=wv_sb, in_=wvT_d[:].rearrange("(dt p) e -> p dt e", p=P))
            nc.sync.dma_start(out=bk_sb, in_=bk_d[:])
            nc.sync.dma_start(out=bq_sb, in_=bq_d[:])
            nc.sync.dma_start(out=wk_sb[:, :, P:E],
                              in_=wkT_d[:, P:E].rearrange("(dt p) e -> p dt e", p=P))
            nc.sync.dma_start(out=wq_sb[:, :, P:E],
                              in_=wqT_d[:, P:E].rearrange("(dt p) e -> p dt e", p=P))
            for c in range(1, T // 512):
                for dt in range(8):
                    nc.sync.dma_start(
                        out=xT_sb[:, dt, c * 512:(c + 1) * 512],
                        in_=xT_d[dt * P:(dt + 1) * P, c * 512:(c + 1) * 512],
                    )
            nc.sync.dma_start(out=id_sb, in_=id_d[:])
            nc.sync.dma_start(out=wo_sb, in_=wo_d[:].rearrange("(m p) f -> p m f", p=P))

            # ones columns of V (denominator rows of the pv matmuls)
            nc.vector.memset(V[:, :, :, 64:65], 1.0)
            nc.vector.memset(ones_bc, 1.0)

            # ---- drip-work helpers (each item is an atomic closure) ----
            def proj_half(w_sb, em, c, dst, b_sb, state, second):
                # half of a K/Q projection chunk: 4 matmuls (+ bias drain)
                def f():
                    if not second:
                        state["ps"] = ps.tile([P, 512], F32, tag="y", bufs=2,
                                              name="proj_ps")
                    for dt in (range(4, 8) if second else range(4)):
                        nc.tensor.matmul(
                            state["ps"],
                            lhsT=w_sb[:, dt, em * P:(em + 1) * P],
                            rhs=xT_sb[:, dt, c * 512:(c + 1) * 512],
                            start=(dt == 0),
                            stop=(dt == 7),
                            skip_group_check=True,
                        )
                    if second:
                        nc.vector.tensor_scalar_add(
                            out=dst[:, em, c * 512:(c + 1) * 512],
                            in0=state["ps"],
                            scalar1=b_sb[:, em:em + 1],
                        )
                return f

            def proj_chunk(w_sb, em, c, dst, b_sb):
                st = {}
                return [proj_half(w_sb, em, c, dst, b_sb, st, False),
                        proj_half(w_sb, em, c, dst, b_sb, st, True)]

            def v_chunk(st):
                # V projection for one 128-wide s-tile (all 4 heads)
                vps = ps.tile([P, 512], F32, tag="y", bufs=2, name="vps")
                for dt in range(8):
                    nc.tensor.matmul(
                        vps[:, :E],
                        lhsT=xT_sb[:, dt, st * P:(st + 1) * P],
                        rhs=wv_sb[:, dt, :],
                        start=(dt == 0),
                        stop=(dt == 7),
                        skip_group_check=True,
                    )
                nc.vector.tensor_copy(out=V[:, st, :, 0:64], in_=vps[:, :E])

            yA_ring = {}

            def y_half(tt, fb, em):
                # Every y unit is em-split: the em0 half (heads 0,1) runs
                # right after that head-pair's normalize — one pass before
                # em1 — parking its partial in a bf16 ring. This turns the
                # bursty 8-unit-per-2-passes y supply into a smooth 8 halves
                # per pass, and leaves only the small em1 half of the last
                # t-block trailing the final softmax bounce.
                def f():
                    yp = ps.tile([P, 512], F32, tag="y", bufs=2, name="y_ps")
                    nc.tensor.matmul(
                        yp,
                        lhsT=outT[:, em, tt * P:(tt + 1) * P],
                        rhs=wo_sb[:, em, fb * 512:(fb + 1) * 512],
                        start=True,
                        stop=True,
                        skip_group_check=True,
                    )
                    if em == 0:
                        ya = sb.tile([P, 512], BF16, tag="yA", bufs=12, name="ya")
                        nc.vector.tensor_copy(out=ya, in_=yp)
                        yA_ring[(tt, fb)] = ya
                    else:
                        ysb = sb.tile([P, 512], F32, tag="ysb", bufs=2, name="ysb")
                        nc.vector.tensor_add(ysb, yp, yA_ring.pop((tt, fb)))
                        nc.sync.dma_start(
                            out=y_d[tt * P:(tt + 1) * P, fb * 512:(fb + 1) * 512],
                            in_=ysb,
                        )
                return f

            def pump(q, n):
                for _ in range(n):
                    if not q:
                        return
                    q.popleft()()

            # ---- phase 1A: first K-em0 and Q-em0 chunks only; all other
            # projection chunks stream into the early passes just ahead of
            # their first use (deadlines noted) ----
            for f in proj_chunk(wk_sb, 0, 0, KT, bk_sb):
                f()
            for f in proj_chunk(wq_sb, 0, 0, QT, bq_sb):
                f()

            projq = deque()
            pass_proj = {
                0: (proj_chunk(wk_sb, 0, 1, KT, bk_sb)      # sc(p0,st4)
                    + proj_chunk(wq_sb, 1, 0, QT, bq_sb)    # p1
                    + proj_chunk(wk_sb, 0, 2, KT, bk_sb)    # sc(p0,st8)
                    + proj_chunk(wk_sb, 1, 0, KT, bk_sb)    # sc(p1,st0)
                    + proj_chunk(wk_sb, 0, 3, KT, bk_sb)),  # sc(p0,st12)
                1: (proj_chunk(wk_sb, 1, 1, KT, bk_sb)      # sc(p1,st4) JIT
                    + proj_chunk(wk_sb, 1, 2, KT, bk_sb)    # sc(p1,st8)
                    + proj_chunk(wk_sb, 1, 3, KT, bk_sb)    # sc(p1,st12)
                    + proj_chunk(wq_sb, 0, 1, QT, bq_sb)),  # p2
                2: proj_chunk(wq_sb, 1, 1, QT, bq_sb),      # p3
                3: proj_chunk(wq_sb, 0, 2, QT, bq_sb),      # p4
                4: proj_chunk(wq_sb, 1, 2, QT, bq_sb),      # p5
                5: proj_chunk(wq_sb, 0, 3, QT, bq_sb),      # p6
                6: proj_chunk(wq_sb, 1, 3, QT, bq_sb),      # p7
            }

            yq = deque()

            def emit_pv(info, st):
                for j in range(2):
                    nc.tensor.matmul(
                        info["pv"][0:65, j, :],
                        lhsT=V[:, st, 2 * info["hp"] + j, 0:65],
                        rhs=info["pT"][:, st, j, :],
                        start=(st == 0),
                        stop=(st == NT - 1),
                        skip_group_check=True,
                    )

            def norm(info):
                # 1/denominator rows, partition-broadcast by a K'=1 PE
                # ones-matmul into a psum bank — no DMA in the normalize
                # chain, so the next pass's pv never waits on a bounce
                rec = sb.tile([65, 2, TB], F32R, tag="rec", bufs=2, name="rec")
                with nc.allow_low_precision(reason="f32r view of f32 reciprocal"):
                    nc.vector.reciprocal(out=rec[64:65, :, :],
                                         in_=info["pv"][64:65, :, :])
                bcs = []
                for j in range(2):
                    bc = ps.tile([P, TB], F32, tag="y", bufs=2, name="bc_ps")
                    nc.tensor.matmul(
                        bc[0:DK, :],
                        lhsT=ones_bc[64:65, :],
                        rhs=rec[64:65, j, :],
                        start=True,
                        stop=True,
                        skip_group_check=True,
                    )
                    bcs.append(bc)
                info["bc"] = bcs

            def mul(info):
                t0 = info["t0"]
                # even head: normalize psum straight into outT rows 0-63
                nc.vector.tensor_mul(
                    outT[0:DK, info["hp"], t0:t0 + TB],
                    info["pv"][0:DK, 0, :],
                    info["bc"][0][0:DK, :],
                )
                # odd head: normalize into SBUF, then PE identity-shift to
                # outT rows 64-127 (DVE lanes can't cross partitions)
                ou = sb.tile([DK, TB], BF16, tag="ou", bufs=2, name="ou")
                nc.vector.tensor_mul(ou, info["pv"][0:DK, 1, :], info["bc"][1][0:DK, :])
                sh = ps.tile([P, TB], F32, tag="y", bufs=2, name="sh_ps")
                nc.tensor.matmul(sh[DK:P, :], lhsT=id_sb, rhs=ou,
                                 start=True, stop=True, skip_group_check=True)
                nc.vector.tensor_copy(
                    out=outT[DK:P, info["hp"], t0:t0 + TB], in_=sh[DK:P, :])

            # ---- main passes ----
            # Emission order per slot: exp(st) first, then pv/spill/drip
            # (ready PE work), then sc(st+1) LAST — the next slot's scores
            # matmul is the only instruction that may briefly block on the
            # exp double-buffer WAR, and putting it after the drip keeps the
            # strict-FIFO PE queue from stalling on it while ready work sits
            # behind it.
            def emit_sc(info, st):
                scp = ps.tile([P, 2, TB], F32, tag="sc", bufs=2, name="sc_ps")
                for j in range(2):
                    nc.tensor.matmul(
                        scp[:, j, :],
                        lhsT=KT[DK * j:DK * (j + 1), info["hp"],
                                st * P:(st + 1) * P],
                        rhs=QT[DK * j:DK * (j + 1), info["hp"],
                               info["t0"]:info["t0"] + TB],
                        start=True,
                        stop=True,
                    )
                info["sc"][st] = scp

            def append_y(info):
                # called right after mul(info): that pass's heads are now
                # normalized in outT
                for tt in range(info["tb"] * (TB // P),
                                (info["tb"] + 1) * (TB // P)):
                    for fb in range(2):
                        yq.append(y_half(tt, fb, info["hp"]))

            prev = None
            for p in range(NPASS + 1):
                last = p == NPASS
                if not last:
                    tb, hp = p // 2, p % 2  # t-block-major: y work starts early
                    cur = {
                        "p": p, "hp": hp, "tb": tb, "t0": tb * TB,
                        "pT": sb.tile([P, NT, 2, TB], BF16, tag="pT", bufs=2,
                                      name="pT"),
                        "pv": ps.tile([65, 2, TB], F32, tag="pv", bufs=1,
                                      name="pv_ps"),
                        "pending": deque(range(NT - LAG, NT)),
                        "sc": {},
                    }
                    projq.extend(pass_proj.get(p, []))
                    emit_sc(cur, 0)

                for st in range(0 if last else NT):
                    nc.scalar.activation(
                        out=cur["pT"][:, st, :, :],
                        in_=cur["sc"].pop(st),
                        func=mybir.ActivationFunctionType.Exp,
                        scale=0.125,
                    )
                    if st + 1 < NT:
                        emit_sc(cur, st + 1)
                    if p == 0:
                        v_chunk(st)
                    if prev is not None:
                        if st <= 1:  # spill: previous pass's lagged pv tail
                            for _ in range(3):
                                if prev["pending"]:
                                    emit_pv(prev, prev["pending"].popleft())
                        elif st == 2:
                            norm(prev)
                        elif st == 4:
                            mul(prev)
                            append_y(prev)
                    if st >= LAG:
                        emit_pv(cur, st - LAG)
                    # drip pacing: passes 0-1 pump projections every slot
                    # (their queues carry K chunks with hard in-pass
                    # deadlines); later passes pump their single chunk at
                    # fixed slots. y halves fill the pv-free slots from the
                    # standing backlog, under-pumped on the final pass so a
                    # backlog rides into the tail window.
                    if p <= 1 or st in (6, 13):
                        pump(projq, 1)
                    if p >= 2 and (st in (3, 5, 6) or
                                   (st % 2 == 1 and p != NPASS - 1)):
                        pump(yq, 1)

                if last:
                    # tail: finish pass 7's pv and get the denominator
                    # bounce into the DMA queue BEFORE the y-backlog's
                    # output DMAs, then burn the backlog during the bounce
                    while prev["pending"]:
                        emit_pv(prev, prev["pending"].popleft())
                    norm(prev)
                    pump(yq, len(yq))  # ready leftovers cover the bounce
                    mul(prev)
                    append_y(prev)
                    pump(projq, len(projq))
                    pump(yq, len(yq))
                else:
                    prev = cur
    _split_multi_waits(nc)
    return nc


def _shard_inputs(x, w_q, b_q, w_k, b_k, w_v, b_v, w_o, b_o):
    from ml_dtypes import bfloat16

    def bf(a):
        return np.ascontiguousarray(a).astype(bfloat16)

    in_maps = []
    for c in range(N_CORES):
        b, g = c // 4, c % 4
        sl = slice(g * E, (g + 1) * E)
        in_maps.append({
            "ident": np.eye(64, dtype=bfloat16),
            "xT": bf(x[b].T),
            "wqT": bf(w_q[sl, :].T),
            "wkT": bf(w_k[sl, :].T),
            "wvT": bf(w_v[sl, :].T),
            "wo_sh": bf(w_o[:, sl].T),
            "bq2": np.ascontiguousarray(b_q[sl].reshape(2, P).T, dtype=np.float32),
            "bk2": np.ascontiguousarray(b_k[sl].reshape(2, P).T, dtype=np.float32),
        })
    return in_maps


_NC_CACHE = {}


def kernel(x, w_q, b_q, w_k, b_k, w_v, b_v, w_o, b_o, _trace=False):
    x = np.asarray(x, dtype=np.float32)
    B, T, _ = x.shape
    args = [np.asarray(a, dtype=np.float32)
            for a in (w_q, b_q, w_k, b_k, w_v, b_v, w_o, b_o)]
    w_q, b_q, w_k, b_k, w_v, b_v, w_o, b_o = args

    if T not in _NC_CACHE:
        _NC_CACHE[T] = build_nc(T=T)
    nc = _NC_CACHE[T]
    in_maps = _shard_inputs(x, w_q, b_q, w_k, b_k, w_v, b_v, w_o, b_o)
    res = run_bass_kernel_spmd(nc, in_maps, list(range(N_CORES)), trace=_trace)

    y = np.zeros((B, T, D), dtype=np.float32)
    for c in range(N_CORES):
        y[c // 4] += res.results[c]["y"]
    fold = b_v @ w_o.T + b_o
    y += fold[None, None, :]
    if _trace:
        return y, res
    return y
